# revision 31
# baseline (speedup 1.0000x reference)
"""Trainium2 Bass kernel for nn_Block_49624052138029 (dense transformer block).

Strategy: pure data parallelism across 8 NeuronCores. Core i handles batch
b = i//4 and query-chunk c = i%4 (512 of the 2048 tokens). The host permutes
each core's batch slice so its own 512 query rows come first; attention is
permutation-invariant over keys, so K/V row order doesn't matter. Each core
redundantly computes LN1 + K + V over all 2048 rows of its batch (cheaper
than on-chip collectives on this stack), and Q/attention/Wo/FFN only for its
own 512 rows.

On-chip layout: activations live feature-major ([D on partitions, tokens on
free]) for matmuls; LayerNorm runs row-major and the gamma/beta application is
fused into the PSUM-evacuation of the PE transpose (where D sits on
partitions). Attention computes S^T = K_h^T-chunks.T @ Q_h^T per head with an
exp() evacuation on ScalarE, and AV appends an all-ones column to V so the
softmax denominator falls out of the same accumulation (row 64 of U^T).

All matmul operands are bf16 (f32 PSUM accumulation); LN statistics,
residual stream and softmax denominators stay f32.
"""

import os
import sys

for _p in ("/root/.axon_site", "/root/.axon_site/_ro/trn_rl_repo",
           "/root/.axon_site/_ro/pypackages", "/opt/trn_rl_repo", "/opt/pypackages"):
    if os.path.isdir(_p) and _p not in sys.path:
        sys.path.append(_p)

import numpy as np
import ml_dtypes

import concourse.bass as bass
import concourse.tile as tile
from concourse import bacc, mybir
from concourse.bass_utils import run_bass_kernel_spmd
from concourse.masks import make_identity

F32 = mybir.dt.float32
BF16 = mybir.dt.bfloat16
AF = mybir.ActivationFunctionType
ALU = mybir.AluOpType
AX = mybir.AxisListType

D = 1024          # model dim
H = 16            # heads
E = 64            # head dim
T = 2048          # tokens per batch
TQ = 512          # tokens owned by this core
P = 128
KO = D // P       # 8 feature chunks
RT_ALL = T // P   # 16 row tiles per batch
RT_OWN = TQ // P  # 4 row tiles owned
SO = T // P       # 16 key chunks
EPS = 1e-5
SCALE = 1.0 / 32.0  # D ** -0.5


def _layer_norm_rows(nc, sng, xrow_ap, stats_pool, y_out_ap):
    """Row-major LayerNorm core: y = (x - mean(x)) * rsqrt(var(x) + eps).

    xrow_ap: [128, 1024] f32 SBUF; y_out_ap: [128, 1024] (any dtype) SBUF.
    gamma/beta are NOT applied here (folded into the transpose evacuation).
    """
    stats = stats_pool.tile([P, 2, 6], F32, tag="bnstats")
    xg = xrow_ap.rearrange("p (g d) -> p g d", g=2)
    for g in range(2):
        nc.vector.bn_stats(out=stats[:, g, :], in_=xg[:, g, :])
    mv = stats_pool.tile([P, 2], F32, tag="bnaggr")
    nc.vector.bn_aggr(out=mv[:], in_=stats[:])
    rstd = stats_pool.tile([P, 1], F32, tag="rstd")
    # rstd = 1 / sqrt(var + eps)
    nc.scalar.activation(out=rstd[:], in_=mv[:, 1:2], func=AF.Sqrt,
                         bias=sng["eps"][:], scale=1.0)
    nc.vector.reciprocal(out=rstd[:], in_=rstd[:])
    nc.vector.tensor_scalar(
        out=y_out_ap, in0=xrow_ap, scalar1=mv[:, 0:1], scalar2=rstd[:],
        op0=ALU.subtract, op1=ALU.mult)


def build_kernel():
    nc = bacc.Bacc(None, target_bir_lowering=False, debug=False)

    xb = nc.dram_tensor("xb", [T, D], F32, kind="ExternalInput")
    wq = nc.dram_tensor("wq", [D, D], BF16, kind="ExternalInput")
    wk = nc.dram_tensor("wk", [D, D], BF16, kind="ExternalInput")
    wv = nc.dram_tensor("wv", [D, D], BF16, kind="ExternalInput")
    wo = nc.dram_tensor("wo", [D, D], BF16, kind="ExternalInput")
    w1 = nc.dram_tensor("w1", [D, D], BF16, kind="ExternalInput")
    w2 = nc.dram_tensor("w2", [D, D], BF16, kind="ExternalInput")
    cq = nc.dram_tensor("cq", [D], F32, kind="ExternalInput")
    ck = nc.dram_tensor("ck", [D], F32, kind="ExternalInput")
    bo = nc.dram_tensor("bo", [D], F32, kind="ExternalInput")
    b1 = nc.dram_tensor("b1", [D], F32, kind="ExternalInput")
    b2 = nc.dram_tensor("b2", [D], F32, kind="ExternalInput")
    out = nc.dram_tensor("out", [TQ, D], F32, kind="ExternalOutput")

    # per-feature params as [128, 8] (partition p, chunk o) for feature-major use
    def pm(dram_vec):
        return dram_vec.rearrange("(o p) -> p o", p=P)

    with tile.TileContext(nc) as tc:
        with (
            tc.tile_pool(name="singles", bufs=1) as singles,
            tc.tile_pool(name="persist", bufs=1) as persist,
            tc.tile_pool(name="hrow", bufs=3) as hrow_pool,
            tc.tile_pool(name="stats", bufs=4) as stats_pool,
            tc.tile_pool(name="wstrip", bufs=2) as wstrip_pool,
        ):
            # ---------------- setup ----------------
            sng = {}
            id_bf = singles.tile([P, P], BF16, name="id_bf")
            make_identity(nc, id_bf[:])
            id_f32 = singles.tile([P, P], F32, name="id_f32")
            make_identity(nc, id_f32[:])
            sng["eps"] = singles.tile([P, 1], F32, name="eps")
            nc.vector.memset(sng["eps"][:], EPS)
            ones64 = singles.tile([1, E], BF16, name="ones64")
            nc.vector.memset(ones64[:], 1.0)


            cq_pm = singles.tile([P, KO], F32, name="cq_pm")
            nc.sync.dma_start(cq_pm[:], pm(cq))
            ck_pm = singles.tile([P, KO], F32, name="ck_pm")
            nc.sync.dma_start(ck_pm[:], pm(ck))
            bo_pm = singles.tile([P, KO], F32, name="bo_pm")
            nc.sync.dma_start(bo_pm[:], pm(bo))
            bf1_pm = singles.tile([P, KO], F32, name="bf1_pm")
            nc.sync.dma_start(bf1_pm[:], pm(b1))
            bf2_pm = singles.tile([P, KO], F32, name="bf2_pm")
            nc.sync.dma_start(bf2_pm[:], pm(b2))

            # ---------------- persistent activations ----------------
            kT = persist.tile([P, KO, T], BF16, name="kT")          # 4 MB
            vP = persist.tile([P, SO, H, E + 1], BF16, name="vP")   # 4.16 MB
            qT = persist.tile([P, KO, TQ], BF16, name="qT")         # 1 MB
            oT = persist.tile([P, KO, TQ], BF16, name="oT")         # 1 MB
            x1 = persist.tile([P, RT_OWN, D], F32, name="x1")       # 2 MB
            h2T = persist.tile([P, KO, TQ], BF16, name="h2T")       # 1 MB
            fT = persist.tile([P, KO, TQ], BF16, name="fT")         # 1 MB

            # ones column of vP (softmax denominator trick)
            nc.vector.memset(vP[:, :, :, E], 1.0)

            # ---------------- phase 1: LN1 over all rows -> hT ----------------
            ps_w_ctx = tc.tile_pool(name="ps_w", bufs=2, space="PSUM")
            ps_w = ps_w_ctx.__enter__()
            ps_tr_ctx = tc.tile_pool(name="ps_tr", bufs=2, space="PSUM")
            ps_tr = ps_tr_ctx.__enter__()
            ctx12 = tc.tile_pool(name="hTp", bufs=1)
            hT_pool = ctx12.__enter__()
            wsmall_ctx = tc.tile_pool(name="wsmall", bufs=3)
            wsmall_pool = wsmall_ctx.__enter__()
            xrow_ctx = tc.tile_pool(name="xrow", bufs=2)
            xrow_pool = xrow_ctx.__enter__()
            hT = hT_pool.tile([P, KO, T], BF16, name="hT")          # 4 MB
            for rt in range(RT_ALL):
                if rt < RT_OWN:
                    x_t = x1[:, rt, :]   # own rows: keep the raw x for residual
                    nc.sync.dma_start(x_t, xb[rt * P:(rt + 1) * P, :])
                else:
                    x_tile = xrow_pool.tile([P, D], F32, tag="xrow")
                    nc.sync.dma_start(x_tile[:], xb[rt * P:(rt + 1) * P, :])
                    x_t = x_tile[:]
                y_row = hrow_pool.tile([P, D], BF16, tag="hrow")
                _layer_norm_rows(nc, sng, x_t, stats_pool, y_row[:])
                # gamma/beta are folded into the weights host-side, so the
                # transpose evacuation is a plain copy (batched 2 chunks/op)
                trp = ps_tr.tile([P, KO, P], BF16, tag="tr")
                for ko in range(KO):
                    nc.tensor.transpose(trp[:, ko, :], y_row[:, ko * P:(ko + 1) * P], id_bf[:])
                nc.scalar.copy(out=hT[:, :, rt * P:(rt + 1) * P], in_=trp[:])

            # ---------------- phases 2+3: q/k/v interleaved with attention ----------------
            xrow_ctx.__exit__(None, None, None)
            ps_tr_ctx.__exit__(None, None, None)
            exps_ctx = tc.tile_pool(name="exps", bufs=6)
            exps_pool = exps_ctx.__enter__()
            evac_ctx = tc.tile_pool(name="evac", bufs=3)
            evac_pool = evac_ctx.__enter__()
            ps_qk_ctx = tc.tile_pool(name="ps_qk", bufs=2, space="PSUM")
            ps_qk = ps_qk_ctx.__enter__()
            ps_u_ctx = tc.tile_pool(name="ps_u", bufs=2, space="PSUM")
            ps_u = ps_u_ctx.__enter__()

            # Dense k/v matmuls are emitted right after each attention head
            # pair (lower scheduler priority), so the in-order PE fills
            # exp-latency gaps with dense work instead of idling.
            def emit_k(he):
                wk_he = wsmall_pool.tile([P, KO, P], BF16, tag="w_he", name="wk_he")
                nc.sync.dma_start(
                    wk_he[:], wk.rearrange("(o p) n -> p o n", p=P)[:, :, he * P:(he + 1) * P])
                for t in range(RT_ALL // 4):
                    psk = ps_w.tile([P, 512], F32, tag="ps_w", name="psk")
                    for ko in range(KO):
                        nc.tensor.matmul(
                            psk[:], wk_he[:, ko, :], hT[:, ko, t * 512:(t + 1) * 512],
                            start=(ko == 0), stop=(ko == KO - 1))
                    nc.vector.tensor_scalar_add(
                        out=kT[:, he, t * 512:(t + 1) * 512], in0=psk[:],
                        scalar1=ck_pm[:, he:he + 1])

            def emit_q(he):
                wq_he = wsmall_pool.tile([P, KO, P], BF16, tag="w_he", name="wq_he")
                nc.sync.dma_start(
                    wq_he[:], wq.rearrange("(o p) n -> p o n", p=P)[:, :, he * P:(he + 1) * P])
                psq = ps_w.tile([P, 512], F32, tag="ps_w", name="psq")
                for ko in range(KO):
                    nc.tensor.matmul(
                        psq[:], wq_he[:, ko, :], hT[:, ko, 0:TQ],
                        start=(ko == 0), stop=(ko == KO - 1))
                nc.vector.tensor_scalar_add(
                    out=qT[:, he, :], in0=psq[:], scalar1=cq_pm[:, he:he + 1])

            def emit_v_load(nh):
                wv_strip = wstrip_pool.tile([P, KO, 512], BF16, tag="wstrip", name="wv_strip")
                nc.sync.dma_start(
                    wv_strip[:], wv.rearrange("(o p) n -> p o n", p=P)[:, :, nh * 512:(nh + 1) * 512])
                return wv_strip

            def emit_v(nh, wv_strip=None, so_list=None):
                if wv_strip is None:
                    wv_strip = emit_v_load(nh)
                for so in (range(SO) if so_list is None else so_list):
                    psv = ps_w.tile([P, 512], F32, tag="ps_w", name="psv")
                    for ko in range(KO):
                        nc.tensor.matmul(
                            psv[:], hT[:, ko, so * P:(so + 1) * P], wv_strip[:, ko, :],
                            start=(ko == 0), stop=(ko == KO - 1))
                    nc.vector.tensor_copy(
                        out=vP[:, so, nh * 8:(nh + 1) * 8, 0:E],
                        in_=psv[:].rearrange("p (h e) -> p h e", e=E))

            def emit_attn(h):
                pbase = (h % 2) * E
                ko_h = h // 2
                psu = ps_u.tile([P, 512], F32, tag="ps_u", name="psu")
                for sp in range(SO // 2):
                    pss = ps_qk.tile([P, 2, 512], F32, tag="ps_qk", name="pss")
                    for j in range(2):
                        so = 2 * sp + j
                        nc.tensor.matmul(
                            pss[:, j, :],
                            kT[pbase:pbase + E, ko_h, so * P:(so + 1) * P],
                            qT[pbase:pbase + E, ko_h, :],
                            start=True, stop=True)
                    es = exps_pool.tile([P, 2, 512], BF16, tag="exps", name="es")
                    nc.scalar.activation(out=es[:], in_=pss[:], func=AF.Exp, scale=SCALE)
                    for j in range(2):
                        so = 2 * sp + j
                        nc.tensor.matmul(
                            psu[0:E + 1, :], vP[:, so, h, :], es[:, j, :],
                            start=(so == 0), stop=(so == SO - 1))
                dinv_f = stats_pool.tile([1, TQ], F32, tag="dinv_f", name="dinv_f")
                nc.vector.reciprocal(out=dinv_f[:], in_=psu[E:E + 1, :])
                dinv = stats_pool.tile([1, TQ], BF16, tag="dinv", name="dinv")
                nc.vector.tensor_copy(out=dinv[:], in_=dinv_f[:])
                psb = ps_u.tile([P, 512], F32, tag="ps_u", name="psb")
                nc.tensor.matmul(psb[0:E, :], ones64[:], dinv[:], start=True, stop=True)
                dbc = evac_pool.tile([E, 512], BF16, tag="dbc", name="dbc")
                nc.vector.tensor_copy(out=dbc[:], in_=psb[0:E, :])
                nc.vector.tensor_tensor(
                    out=oT[pbase:pbase + E, ko_h, :], in0=psu[0:E, :], in1=dbc[:],
                    op=ALU.mult)

            for he in range(KO):
                emit_q(he)
            emit_k(0)
            emit_v(0)
            for pair in range(KO):
                emit_attn(2 * pair)
                if pair + 1 < KO:
                    emit_k(pair + 1)
                emit_attn(2 * pair + 1)
                if pair == 2:
                    v1_strip = emit_v_load(1)
                    emit_v(1, v1_strip, list(range(0, 8)))
                elif pair == 3:
                    emit_v(1, v1_strip, list(range(8, SO)))


            ps_u_ctx.__exit__(None, None, None)
            ps_qk_ctx.__exit__(None, None, None)
            evac_ctx.__exit__(None, None, None)
            exps_ctx.__exit__(None, None, None)
            wsmall_ctx.__exit__(None, None, None)
            ctx12.__exit__(None, None, None)
            evac_ctx = tc.tile_pool(name="evac2", bufs=3)
            evac_pool = evac_ctx.__enter__()
            ps_tr_ctx = tc.tile_pool(name="ps_tr2", bufs=2, space="PSUM")
            ps_tr = ps_tr_ctx.__enter__()

            # ---------------- phase 4: Wo projection + residual + LN2 ----------------
            for half in range(2):
                wo_strip = wstrip_pool.tile([P, KO, 512], BF16, tag="wstrip")
                nc.sync.dma_start(
                    wo_strip[:], wo.rearrange("(o p) n -> p o n", p=P)[:, :, half * 512:(half + 1) * 512])
                for m in range(4):
                    mm = half * 4 + m
                    psy = ps_w.tile([P, 512], F32, tag="ps_w")
                    for ko in range(KO):
                        nc.tensor.matmul(
                            psy[:], wo_strip[:, ko, m * P:(m + 1) * P], oT[:, ko, :],
                            start=(ko == 0), stop=(ko == KO - 1))
                    ysb = evac_pool.tile([P, 512], F32, tag="ysb")
                    nc.vector.tensor_scalar_add(out=ysb[:], in0=psy[:], scalar1=bo_pm[:, mm:mm + 1])
                    trp = ps_tr.tile([P, RT_OWN, P], F32, tag="tr")
                    for rt in range(RT_OWN):
                        nc.tensor.transpose(trp[:, rt, :], ysb[:, rt * P:(rt + 1) * P], id_f32[:])
                    nc.vector.tensor_tensor(
                        out=x1[:, :, mm * P:(mm + 1) * P],
                        in0=x1[:, :, mm * P:(mm + 1) * P], in1=trp[:], op=ALU.add)

            for rt in range(RT_OWN):
                y_row = hrow_pool.tile([P, D], BF16, tag="hrow")
                _layer_norm_rows(nc, sng, x1[:, rt, :], stats_pool, y_row[:])
                trp = ps_tr.tile([P, KO, P], BF16, tag="tr2")
                for ko in range(KO):
                    nc.tensor.transpose(trp[:, ko, :], y_row[:, ko * P:(ko + 1) * P], id_bf[:])
                nc.scalar.copy(out=h2T[:, :, rt * P:(rt + 1) * P], in_=trp[:])

            # ---------------- phase 5: FFN ----------------
            for half in range(2):
                w1_strip = wstrip_pool.tile([P, KO, 512], BF16, tag="wstrip")
                nc.sync.dma_start(
                    w1_strip[:], w1.rearrange("(o p) n -> p o n", p=P)[:, :, half * 512:(half + 1) * 512])
                for m in range(4):
                    mm = half * 4 + m
                    psf = ps_w.tile([P, 512], F32, tag="ps_w")
                    for ko in range(KO):
                        nc.tensor.matmul(
                            psf[:], w1_strip[:, ko, m * P:(m + 1) * P], h2T[:, ko, :],
                            start=(ko == 0), stop=(ko == KO - 1))
                    # f = gelu(x + b1), fused bias via activation
                    nc.scalar.activation(out=fT[:, mm, :], in_=psf[:], func=AF.Gelu,
                                         bias=bf1_pm[:, mm:mm + 1], scale=1.0)
            for half in range(2):
                w2_strip = wstrip_pool.tile([P, KO, 512], BF16, tag="wstrip")
                nc.sync.dma_start(
                    w2_strip[:], w2.rearrange("(o p) n -> p o n", p=P)[:, :, half * 512:(half + 1) * 512])
                for m in range(4):
                    mm = half * 4 + m
                    psz = ps_w.tile([P, 512], F32, tag="ps_w")
                    for ko in range(KO):
                        nc.tensor.matmul(
                            psz[:], w2_strip[:, ko, m * P:(m + 1) * P], fT[:, ko, :],
                            start=(ko == 0), stop=(ko == KO - 1))
                    zsb = evac_pool.tile([P, 512], F32, tag="ysb")
                    nc.vector.tensor_scalar_add(out=zsb[:], in0=psz[:], scalar1=bf2_pm[:, mm:mm + 1])
                    trp = ps_tr.tile([P, RT_OWN, P], F32, tag="tr")
                    for rt in range(RT_OWN):
                        nc.tensor.transpose(trp[:, rt, :], zsb[:, rt * P:(rt + 1) * P], id_f32[:])
                    nc.vector.tensor_tensor(
                        out=x1[:, :, mm * P:(mm + 1) * P],
                        in0=x1[:, :, mm * P:(mm + 1) * P], in1=trp[:], op=ALU.add)

            for rt in range(RT_OWN):
                nc.sync.dma_start(out[rt * P:(rt + 1) * P, :], x1[:, rt, :])

            ps_tr_ctx.__exit__(None, None, None)
            evac_ctx.__exit__(None, None, None)
            ps_w_ctx.__exit__(None, None, None)

    nc.compile()
    return nc


_NC_CACHE = None


def _get_nc():
    global _NC_CACHE
    if _NC_CACHE is None:
        _NC_CACHE = build_kernel()
    return _NC_CACHE


def _prep_weights(Wq, Wk, Wv, Wo, W1, W2, ln1_g, ln1_b, ln2_g, ln2_b, b1):
    """Fold LayerNorm gamma into the consuming weights and beta into bias
    vectors (exact math, done in f32 before the bf16 cast)."""
    bf = ml_dtypes.bfloat16
    # [H, D, E] -> [D, H*E]
    wq = np.ascontiguousarray(np.transpose(Wq, (1, 0, 2)).reshape(D, D))
    wk = np.ascontiguousarray(np.transpose(Wk, (1, 0, 2)).reshape(D, D))
    wv = np.ascontiguousarray(np.transpose(Wv, (1, 0, 2)).reshape(D, D))
    cq = ln1_b @ wq
    ck = ln1_b @ wk
    cv = ln1_b @ wv              # v bias; o = softmax(..)@v + cv, folded into bo
    bo_adj = cv @ Wo             # caller adds this to bo
    b1_adj = b1 + ln2_b @ W1
    return ((wq * ln1_g[:, None]).astype(bf), (wk * ln1_g[:, None]).astype(bf),
            (wv * ln1_g[:, None]).astype(bf), Wo.astype(bf),
            (W1 * ln2_g[:, None]).astype(bf), W2.astype(bf),
            cq.astype(np.float32), ck.astype(np.float32),
            bo_adj.astype(np.float32), b1_adj.astype(np.float32))


def kernel(x, Wq, Wk, Wv, Wo, bo, ln1_g, ln1_b, ln2_g, ln2_b, W1, b1, W2, b2,
           _trace=False):
    x = np.asarray(x, dtype=np.float32)
    wq, wk, wv, wo, w1, w2, cq_v, ck_v, bo_extra, b1_adj = _prep_weights(
        np.asarray(Wq, np.float32), np.asarray(Wk, np.float32),
        np.asarray(Wv, np.float32), np.asarray(Wo, np.float32),
        np.asarray(W1, np.float32), np.asarray(W2, np.float32),
        np.asarray(ln1_g, np.float32), np.asarray(ln1_b, np.float32),
        np.asarray(ln2_g, np.float32), np.asarray(ln2_b, np.float32),
        np.asarray(b1, np.float32))
    common = {
        "wq": wq, "wk": wk, "wv": wv, "wo": wo, "w1": w1, "w2": w2,
        "cq": cq_v, "ck": ck_v,
        "bo": np.asarray(bo, np.float32) + bo_extra, "b1": b1_adj,
        "b2": np.asarray(b2, np.float32),
    }
    in_maps = []
    for core in range(8):
        b, c = divmod(core, 4)
        xb_perm = np.concatenate(
            [x[b, c * TQ:(c + 1) * TQ], x[b, :c * TQ], x[b, (c + 1) * TQ:]], axis=0)
        in_maps.append({"xb": np.ascontiguousarray(xb_perm), **common})

    nc = _get_nc()
    res = run_bass_kernel_spmd(nc, in_maps, core_ids=list(range(8)), trace=_trace)
    out = np.empty((2, T, D), np.float32)
    for core in range(8):
        b, c = divmod(core, 4)
        out[b, c * TQ:(c + 1) * TQ] = res.results[core]["out"]
    if _trace:
        kernel.last_results = res
    return out


# revision 32
# speedup vs baseline: 1.0093x; 1.0093x over previous
"""Trainium2 Bass kernel for nn_Block_49624052138029 (dense transformer block).

Strategy: pure data parallelism across 8 NeuronCores. Core i handles batch
b = i//4 and query-chunk c = i%4 (512 of the 2048 tokens). The host permutes
each core's batch slice so its own 512 query rows come first; attention is
permutation-invariant over keys, so K/V row order doesn't matter. Each core
redundantly computes LN1 + K + V over all 2048 rows of its batch (cheaper
than on-chip collectives on this stack), and Q/attention/Wo/FFN only for its
own 512 rows.

On-chip layout: activations live feature-major ([D on partitions, tokens on
free]) for matmuls; LayerNorm runs row-major and the gamma/beta application is
fused into the PSUM-evacuation of the PE transpose (where D sits on
partitions). Attention computes S^T = K_h^T-chunks.T @ Q_h^T per head with an
exp() evacuation on ScalarE, and AV appends an all-ones column to V so the
softmax denominator falls out of the same accumulation (row 64 of U^T).

All matmul operands are bf16 (f32 PSUM accumulation); LN statistics,
residual stream and softmax denominators stay f32.
"""

import os
import sys

for _p in ("/root/.axon_site", "/root/.axon_site/_ro/trn_rl_repo",
           "/root/.axon_site/_ro/pypackages", "/opt/trn_rl_repo", "/opt/pypackages"):
    if os.path.isdir(_p) and _p not in sys.path:
        sys.path.append(_p)

import numpy as np
import ml_dtypes

import concourse.bass as bass
import concourse.tile as tile
from concourse import bacc, mybir
from concourse.bass_utils import run_bass_kernel_spmd
from concourse.masks import make_identity

F32 = mybir.dt.float32
BF16 = mybir.dt.bfloat16
FP8 = mybir.dt.float8e4
AF = mybir.ActivationFunctionType
ALU = mybir.AluOpType
AX = mybir.AxisListType

D = 1024          # model dim
H = 16            # heads
E = 64            # head dim
T = 2048          # tokens per batch
TQ = 512          # tokens owned by this core
P = 128
KO = D // P       # 8 feature chunks
RT_ALL = T // P   # 16 row tiles per batch
RT_OWN = TQ // P  # 4 row tiles owned
SO = T // P       # 16 key chunks
EPS = 1e-5
SCALE = 1.0 / 32.0  # D ** -0.5


def _layer_norm_rows(nc, sng, xrow_ap, stats_pool, y_out_ap):
    """Row-major LayerNorm core: y = (x - mean(x)) * rsqrt(var(x) + eps).

    xrow_ap: [128, 1024] f32 SBUF; y_out_ap: [128, 1024] (any dtype) SBUF.
    gamma/beta are NOT applied here (folded into the transpose evacuation).
    """
    stats = stats_pool.tile([P, 2, 6], F32, tag="bnstats")
    xg = xrow_ap.rearrange("p (g d) -> p g d", g=2)
    for g in range(2):
        nc.vector.bn_stats(out=stats[:, g, :], in_=xg[:, g, :])
    mv = stats_pool.tile([P, 2], F32, tag="bnaggr")
    nc.vector.bn_aggr(out=mv[:], in_=stats[:])
    rstd = stats_pool.tile([P, 1], F32, tag="rstd")
    # rstd = 1 / sqrt(var + eps)
    nc.scalar.activation(out=rstd[:], in_=mv[:, 1:2], func=AF.Sqrt,
                         bias=sng["eps"][:], scale=1.0)
    nc.vector.reciprocal(out=rstd[:], in_=rstd[:])
    nc.vector.tensor_scalar(
        out=y_out_ap, in0=xrow_ap, scalar1=mv[:, 0:1], scalar2=rstd[:],
        op0=ALU.subtract, op1=ALU.mult)


def build_kernel():
    nc = bacc.Bacc(None, target_bir_lowering=False, debug=False)

    xb = nc.dram_tensor("xb", [T, D], F32, kind="ExternalInput")
    wq = nc.dram_tensor("wq", [D, D], BF16, kind="ExternalInput")
    wk = nc.dram_tensor("wk", [D, D], BF16, kind="ExternalInput")
    wv = nc.dram_tensor("wv", [D, D], BF16, kind="ExternalInput")
    wo = nc.dram_tensor("wo", [D, D], BF16, kind="ExternalInput")
    w1 = nc.dram_tensor("w1", [D, D], BF16, kind="ExternalInput")
    w2 = nc.dram_tensor("w2", [D, D], BF16, kind="ExternalInput")
    cq = nc.dram_tensor("cq", [D], F32, kind="ExternalInput")
    ck = nc.dram_tensor("ck", [D], F32, kind="ExternalInput")
    bo = nc.dram_tensor("bo", [D], F32, kind="ExternalInput")
    b1 = nc.dram_tensor("b1", [D], F32, kind="ExternalInput")
    b2 = nc.dram_tensor("b2", [D], F32, kind="ExternalInput")
    out = nc.dram_tensor("out", [TQ, D], F32, kind="ExternalOutput")

    # per-feature params as [128, 8] (partition p, chunk o) for feature-major use
    def pm(dram_vec):
        return dram_vec.rearrange("(o p) -> p o", p=P)

    with tile.TileContext(nc) as tc:
        with (
            tc.tile_pool(name="singles", bufs=1) as singles,
            tc.tile_pool(name="persist", bufs=1) as persist,
            tc.tile_pool(name="hrow", bufs=3) as hrow_pool,
            tc.tile_pool(name="stats", bufs=4) as stats_pool,
            tc.tile_pool(name="wstrip", bufs=2) as wstrip_pool,
        ):
            # ---------------- setup ----------------
            sng = {}
            id_bf = singles.tile([P, P], BF16, name="id_bf")
            make_identity(nc, id_bf[:])
            id_f32 = singles.tile([P, P], F32, name="id_f32")
            make_identity(nc, id_f32[:])
            sng["eps"] = singles.tile([P, 1], F32, name="eps")
            nc.vector.memset(sng["eps"][:], EPS)
            ones64 = singles.tile([1, E], BF16, name="ones64")
            nc.vector.memset(ones64[:], 1.0)


            cq_pm = singles.tile([P, KO], F32, name="cq_pm")
            nc.sync.dma_start(cq_pm[:], pm(cq))
            ck_pm = singles.tile([P, KO], F32, name="ck_pm")
            nc.sync.dma_start(ck_pm[:], pm(ck))
            bo_pm = singles.tile([P, KO], F32, name="bo_pm")
            nc.sync.dma_start(bo_pm[:], pm(bo))
            bf1_pm = singles.tile([P, KO], F32, name="bf1_pm")
            nc.sync.dma_start(bf1_pm[:], pm(b1))
            bf2_pm = singles.tile([P, KO], F32, name="bf2_pm")
            nc.sync.dma_start(bf2_pm[:], pm(b2))

            # ---------------- persistent activations ----------------
            kT = persist.tile([P, KO, T], BF16, name="kT")          # 4 MB
            vP = persist.tile([P, SO // 2, 2, H, E + 1], FP8, name="vP")  # 2.08 MB
            qT = persist.tile([P, KO, TQ], BF16, name="qT")         # 1 MB
            oT = persist.tile([P, KO, TQ], BF16, name="oT")         # 1 MB
            x1 = persist.tile([P, RT_OWN, D], F32, name="x1")       # 2 MB
            h2T = persist.tile([P, KO, TQ], BF16, name="h2T")       # 1 MB
            fT = persist.tile([P, KO, TQ], BF16, name="fT")         # 1 MB

            # ones column of vP (softmax denominator trick)
            nc.vector.memset(vP[:, :, :, :, E], 1.0)

            # ---------------- phase 1: LN1 over all rows -> hT ----------------
            ps_w_ctx = tc.tile_pool(name="ps_w", bufs=2, space="PSUM")
            ps_w = ps_w_ctx.__enter__()
            ps_tr_ctx = tc.tile_pool(name="ps_tr", bufs=2, space="PSUM")
            ps_tr = ps_tr_ctx.__enter__()
            ctx12 = tc.tile_pool(name="hTp", bufs=1)
            hT_pool = ctx12.__enter__()
            wsmall_ctx = tc.tile_pool(name="wsmall", bufs=3)
            wsmall_pool = wsmall_ctx.__enter__()
            xrow_ctx = tc.tile_pool(name="xrow", bufs=2)
            xrow_pool = xrow_ctx.__enter__()
            hT = hT_pool.tile([P, KO, T], BF16, name="hT")          # 4 MB
            for rt in range(RT_ALL):
                if rt < RT_OWN:
                    x_t = x1[:, rt, :]   # own rows: keep the raw x for residual
                    nc.sync.dma_start(x_t, xb[rt * P:(rt + 1) * P, :])
                else:
                    x_tile = xrow_pool.tile([P, D], F32, tag="xrow")
                    nc.sync.dma_start(x_tile[:], xb[rt * P:(rt + 1) * P, :])
                    x_t = x_tile[:]
                y_row = hrow_pool.tile([P, D], BF16, tag="hrow")
                _layer_norm_rows(nc, sng, x_t, stats_pool, y_row[:])
                # gamma/beta are folded into the weights host-side, so the
                # transpose evacuation is a plain copy (batched 2 chunks/op)
                trp = ps_tr.tile([P, KO, P], BF16, tag="tr")
                for ko in range(KO):
                    nc.tensor.transpose(trp[:, ko, :], y_row[:, ko * P:(ko + 1) * P], id_bf[:])
                nc.scalar.copy(out=hT[:, :, rt * P:(rt + 1) * P], in_=trp[:])

            # ---------------- phases 2+3: q/k/v interleaved with attention ----------------
            xrow_ctx.__exit__(None, None, None)
            ps_tr_ctx.__exit__(None, None, None)
            exps_ctx = tc.tile_pool(name="exps", bufs=6)
            exps_pool = exps_ctx.__enter__()
            evac_ctx = tc.tile_pool(name="evac", bufs=3)
            evac_pool = evac_ctx.__enter__()
            ps_qk_ctx = tc.tile_pool(name="ps_qk", bufs=2, space="PSUM")
            ps_qk = ps_qk_ctx.__enter__()
            ps_u_ctx = tc.tile_pool(name="ps_u", bufs=2, space="PSUM")
            ps_u = ps_u_ctx.__enter__()

            # Dense k/v matmuls are emitted right after each attention head
            # pair (lower scheduler priority), so the in-order PE fills
            # exp-latency gaps with dense work instead of idling.
            def emit_k(he):
                wk_he = wsmall_pool.tile([P, KO, P], BF16, tag="w_he", name="wk_he")
                nc.sync.dma_start(
                    wk_he[:], wk.rearrange("(o p) n -> p o n", p=P)[:, :, he * P:(he + 1) * P])
                for t in range(RT_ALL // 4):
                    psk = ps_w.tile([P, 512], F32, tag="ps_w", name="psk")
                    for ko in range(KO):
                        nc.tensor.matmul(
                            psk[:], wk_he[:, ko, :], hT[:, ko, t * 512:(t + 1) * 512],
                            start=(ko == 0), stop=(ko == KO - 1))
                    nc.vector.tensor_scalar_add(
                        out=kT[:, he, t * 512:(t + 1) * 512], in0=psk[:],
                        scalar1=ck_pm[:, he:he + 1])

            def emit_q(he):
                wq_he = wsmall_pool.tile([P, KO, P], BF16, tag="w_he", name="wq_he")
                nc.sync.dma_start(
                    wq_he[:], wq.rearrange("(o p) n -> p o n", p=P)[:, :, he * P:(he + 1) * P])
                psq = ps_w.tile([P, 512], F32, tag="ps_w", name="psq")
                for ko in range(KO):
                    nc.tensor.matmul(
                        psq[:], wq_he[:, ko, :], hT[:, ko, 0:TQ],
                        start=(ko == 0), stop=(ko == KO - 1))
                nc.vector.tensor_scalar_add(
                    out=qT[:, he, :], in0=psq[:], scalar1=cq_pm[:, he:he + 1])

            def emit_v_load(nh):
                wv_strip = wstrip_pool.tile([P, KO, 512], BF16, tag="wstrip", name="wv_strip")
                nc.sync.dma_start(
                    wv_strip[:], wv.rearrange("(o p) n -> p o n", p=P)[:, :, nh * 512:(nh + 1) * 512])
                return wv_strip

            def emit_v(nh, wv_strip=None, so_list=None):
                if wv_strip is None:
                    wv_strip = emit_v_load(nh)
                for so in (range(SO) if so_list is None else so_list):
                    psv = ps_w.tile([P, 512], F32, tag="ps_w", name="psv")
                    for ko in range(KO):
                        nc.tensor.matmul(
                            psv[:], hT[:, ko, so * P:(so + 1) * P], wv_strip[:, ko, :],
                            start=(ko == 0), stop=(ko == KO - 1))
                    nc.vector.tensor_copy(
                        out=vP[:, so // 2, so % 2, nh * 8:(nh + 1) * 8, 0:E],
                        in_=psv[:].rearrange("p (h e) -> p h e", e=E))

            def emit_attn(h):
                pbase = (h % 2) * E
                ko_h = h // 2
                psu = ps_u.tile([P, 512], F32, tag="ps_u", name="psu")
                for sp in range(SO // 2):
                    pss = ps_qk.tile([P, 2, 512], F32, tag="ps_qk", name="pss")
                    for j in range(2):
                        so = 2 * sp + j
                        nc.tensor.matmul(
                            pss[:, j, :],
                            kT[pbase:pbase + E, ko_h, so * P:(so + 1) * P],
                            qT[pbase:pbase + E, ko_h, :],
                            start=True, stop=True)
                    es = exps_pool.tile([P, 2, 512], FP8, tag="exps", name="es")
                    nc.scalar.activation(out=es[:], in_=pss[:], func=AF.Exp, scale=SCALE)
                    # fp8 DoubleRow: virtual K=256 sums both key chunks at once
                    nc.tensor.matmul(
                        psu[0:E + 1, :], vP[:, sp, :, h, :], es[:],
                        start=(sp == 0), stop=(sp == SO // 2 - 1),
                        perf_mode=mybir.MatmulPerfMode.DoubleRow)
                dinv_f = stats_pool.tile([1, TQ], F32, tag="dinv_f", name="dinv_f")
                nc.vector.reciprocal(out=dinv_f[:], in_=psu[E:E + 1, :])
                dinv = stats_pool.tile([1, TQ], BF16, tag="dinv", name="dinv")
                nc.vector.tensor_copy(out=dinv[:], in_=dinv_f[:])
                psb = ps_u.tile([P, 512], F32, tag="ps_u", name="psb")
                nc.tensor.matmul(psb[0:E, :], ones64[:], dinv[:], start=True, stop=True)
                dbc = evac_pool.tile([E, 512], BF16, tag="dbc", name="dbc")
                nc.vector.tensor_copy(out=dbc[:], in_=psb[0:E, :])
                nc.vector.tensor_tensor(
                    out=oT[pbase:pbase + E, ko_h, :], in0=psu[0:E, :], in1=dbc[:],
                    op=ALU.mult)

            for he in range(KO):
                emit_q(he)
            emit_k(0)
            emit_v(0)
            for pair in range(KO):
                emit_attn(2 * pair)
                if pair + 1 < KO:
                    emit_k(pair + 1)
                emit_attn(2 * pair + 1)
                if pair == 2:
                    v1_strip = emit_v_load(1)
                    emit_v(1, v1_strip, list(range(0, 8)))
                elif pair == 3:
                    emit_v(1, v1_strip, list(range(8, SO)))


            ps_u_ctx.__exit__(None, None, None)
            ps_qk_ctx.__exit__(None, None, None)
            evac_ctx.__exit__(None, None, None)
            exps_ctx.__exit__(None, None, None)
            wsmall_ctx.__exit__(None, None, None)
            ctx12.__exit__(None, None, None)
            evac_ctx = tc.tile_pool(name="evac2", bufs=3)
            evac_pool = evac_ctx.__enter__()
            ps_tr_ctx = tc.tile_pool(name="ps_tr2", bufs=2, space="PSUM")
            ps_tr = ps_tr_ctx.__enter__()

            # ---------------- phase 4: Wo projection + residual + LN2 ----------------
            for half in range(2):
                wo_strip = wstrip_pool.tile([P, KO, 512], BF16, tag="wstrip")
                nc.sync.dma_start(
                    wo_strip[:], wo.rearrange("(o p) n -> p o n", p=P)[:, :, half * 512:(half + 1) * 512])
                for m in range(4):
                    mm = half * 4 + m
                    psy = ps_w.tile([P, 512], F32, tag="ps_w")
                    for ko in range(KO):
                        nc.tensor.matmul(
                            psy[:], wo_strip[:, ko, m * P:(m + 1) * P], oT[:, ko, :],
                            start=(ko == 0), stop=(ko == KO - 1))
                    ysb = evac_pool.tile([P, 512], F32, tag="ysb")
                    nc.vector.tensor_scalar_add(out=ysb[:], in0=psy[:], scalar1=bo_pm[:, mm:mm + 1])
                    trp = ps_tr.tile([P, RT_OWN, P], F32, tag="tr")
                    for rt in range(RT_OWN):
                        nc.tensor.transpose(trp[:, rt, :], ysb[:, rt * P:(rt + 1) * P], id_f32[:])
                    nc.vector.tensor_tensor(
                        out=x1[:, :, mm * P:(mm + 1) * P],
                        in0=x1[:, :, mm * P:(mm + 1) * P], in1=trp[:], op=ALU.add)

            for rt in range(RT_OWN):
                y_row = hrow_pool.tile([P, D], BF16, tag="hrow")
                _layer_norm_rows(nc, sng, x1[:, rt, :], stats_pool, y_row[:])
                trp = ps_tr.tile([P, KO, P], BF16, tag="tr2")
                for ko in range(KO):
                    nc.tensor.transpose(trp[:, ko, :], y_row[:, ko * P:(ko + 1) * P], id_bf[:])
                nc.scalar.copy(out=h2T[:, :, rt * P:(rt + 1) * P], in_=trp[:])

            # ---------------- phase 5: FFN ----------------
            for half in range(2):
                w1_strip = wstrip_pool.tile([P, KO, 512], BF16, tag="wstrip")
                nc.sync.dma_start(
                    w1_strip[:], w1.rearrange("(o p) n -> p o n", p=P)[:, :, half * 512:(half + 1) * 512])
                for m in range(4):
                    mm = half * 4 + m
                    psf = ps_w.tile([P, 512], F32, tag="ps_w")
                    for ko in range(KO):
                        nc.tensor.matmul(
                            psf[:], w1_strip[:, ko, m * P:(m + 1) * P], h2T[:, ko, :],
                            start=(ko == 0), stop=(ko == KO - 1))
                    # f = gelu(x + b1), fused bias via activation
                    nc.scalar.activation(out=fT[:, mm, :], in_=psf[:], func=AF.Gelu,
                                         bias=bf1_pm[:, mm:mm + 1], scale=1.0)
            for half in range(2):
                w2_strip = wstrip_pool.tile([P, KO, 512], BF16, tag="wstrip")
                nc.sync.dma_start(
                    w2_strip[:], w2.rearrange("(o p) n -> p o n", p=P)[:, :, half * 512:(half + 1) * 512])
                for m in range(4):
                    mm = half * 4 + m
                    psz = ps_w.tile([P, 512], F32, tag="ps_w")
                    for ko in range(KO):
                        nc.tensor.matmul(
                            psz[:], w2_strip[:, ko, m * P:(m + 1) * P], fT[:, ko, :],
                            start=(ko == 0), stop=(ko == KO - 1))
                    zsb = evac_pool.tile([P, 512], F32, tag="ysb")
                    nc.vector.tensor_scalar_add(out=zsb[:], in0=psz[:], scalar1=bf2_pm[:, mm:mm + 1])
                    trp = ps_tr.tile([P, RT_OWN, P], F32, tag="tr")
                    for rt in range(RT_OWN):
                        nc.tensor.transpose(trp[:, rt, :], zsb[:, rt * P:(rt + 1) * P], id_f32[:])
                    nc.vector.tensor_tensor(
                        out=x1[:, :, mm * P:(mm + 1) * P],
                        in0=x1[:, :, mm * P:(mm + 1) * P], in1=trp[:], op=ALU.add)

            for rt in range(RT_OWN):
                nc.sync.dma_start(out[rt * P:(rt + 1) * P, :], x1[:, rt, :])

            ps_tr_ctx.__exit__(None, None, None)
            evac_ctx.__exit__(None, None, None)
            ps_w_ctx.__exit__(None, None, None)

    nc.compile()
    return nc


_NC_CACHE = None


def _get_nc():
    global _NC_CACHE
    if _NC_CACHE is None:
        _NC_CACHE = build_kernel()
    return _NC_CACHE


def _prep_weights(Wq, Wk, Wv, Wo, W1, W2, ln1_g, ln1_b, ln2_g, ln2_b, b1):
    """Fold LayerNorm gamma into the consuming weights and beta into bias
    vectors (exact math, done in f32 before the bf16 cast)."""
    bf = ml_dtypes.bfloat16
    # [H, D, E] -> [D, H*E]
    wq = np.ascontiguousarray(np.transpose(Wq, (1, 0, 2)).reshape(D, D))
    wk = np.ascontiguousarray(np.transpose(Wk, (1, 0, 2)).reshape(D, D))
    wv = np.ascontiguousarray(np.transpose(Wv, (1, 0, 2)).reshape(D, D))
    cq = ln1_b @ wq
    ck = ln1_b @ wk
    cv = ln1_b @ wv              # v bias; o = softmax(..)@v + cv, folded into bo
    bo_adj = cv @ Wo             # caller adds this to bo
    b1_adj = b1 + ln2_b @ W1
    return ((wq * ln1_g[:, None]).astype(bf), (wk * ln1_g[:, None]).astype(bf),
            (wv * ln1_g[:, None]).astype(bf), Wo.astype(bf),
            (W1 * ln2_g[:, None]).astype(bf), W2.astype(bf),
            cq.astype(np.float32), ck.astype(np.float32),
            bo_adj.astype(np.float32), b1_adj.astype(np.float32))


def kernel(x, Wq, Wk, Wv, Wo, bo, ln1_g, ln1_b, ln2_g, ln2_b, W1, b1, W2, b2,
           _trace=False):
    x = np.asarray(x, dtype=np.float32)
    wq, wk, wv, wo, w1, w2, cq_v, ck_v, bo_extra, b1_adj = _prep_weights(
        np.asarray(Wq, np.float32), np.asarray(Wk, np.float32),
        np.asarray(Wv, np.float32), np.asarray(Wo, np.float32),
        np.asarray(W1, np.float32), np.asarray(W2, np.float32),
        np.asarray(ln1_g, np.float32), np.asarray(ln1_b, np.float32),
        np.asarray(ln2_g, np.float32), np.asarray(ln2_b, np.float32),
        np.asarray(b1, np.float32))
    common = {
        "wq": wq, "wk": wk, "wv": wv, "wo": wo, "w1": w1, "w2": w2,
        "cq": cq_v, "ck": ck_v,
        "bo": np.asarray(bo, np.float32) + bo_extra, "b1": b1_adj,
        "b2": np.asarray(b2, np.float32),
    }
    in_maps = []
    for core in range(8):
        b, c = divmod(core, 4)
        xb_perm = np.concatenate(
            [x[b, c * TQ:(c + 1) * TQ], x[b, :c * TQ], x[b, (c + 1) * TQ:]], axis=0)
        in_maps.append({"xb": np.ascontiguousarray(xb_perm), **common})

    nc = _get_nc()
    res = run_bass_kernel_spmd(nc, in_maps, core_ids=list(range(8)), trace=_trace)
    out = np.empty((2, T, D), np.float32)
    for core in range(8):
        b, c = divmod(core, 4)
        out[b, c * TQ:(c + 1) * TQ] = res.results[core]["out"]
    if _trace:
        kernel.last_results = res
    return out


# revision 33
# speedup vs baseline: 1.1135x; 1.1032x over previous
"""Trainium2 Bass kernel for nn_Block_49624052138029 (dense transformer block).

Strategy: pure data parallelism across 8 NeuronCores. Core i handles batch
b = i//4 and query-chunk c = i%4 (512 of the 2048 tokens). The host permutes
each core's batch slice so its own 512 query rows come first; attention is
permutation-invariant over keys, so K/V row order doesn't matter. Each core
redundantly computes LN1 + K + V over all 2048 rows of its batch (cheaper
than on-chip collectives on this stack), and Q/attention/Wo/FFN only for its
own 512 rows.

On-chip layout: activations live feature-major ([D on partitions, tokens on
free]) for matmuls; LayerNorm runs row-major and the gamma/beta application is
fused into the PSUM-evacuation of the PE transpose (where D sits on
partitions). Attention computes S^T = K_h^T-chunks.T @ Q_h^T per head with an
exp() evacuation on ScalarE, and AV appends an all-ones column to V so the
softmax denominator falls out of the same accumulation (row 64 of U^T).

All matmul operands are bf16 (f32 PSUM accumulation); LN statistics,
residual stream and softmax denominators stay f32.
"""

import os
import sys

for _p in ("/root/.axon_site", "/root/.axon_site/_ro/trn_rl_repo",
           "/root/.axon_site/_ro/pypackages", "/opt/trn_rl_repo", "/opt/pypackages"):
    if os.path.isdir(_p) and _p not in sys.path:
        sys.path.append(_p)

import numpy as np
import ml_dtypes

import concourse.bass as bass
import concourse.tile as tile
from concourse import bacc, mybir
from concourse.bass_utils import run_bass_kernel_spmd
from concourse.masks import make_identity

F32 = mybir.dt.float32
BF16 = mybir.dt.bfloat16
FP8 = mybir.dt.float8e4
AF = mybir.ActivationFunctionType
ALU = mybir.AluOpType
AX = mybir.AxisListType

D = 1024          # model dim
H = 16            # heads
E = 64            # head dim
T = 2048          # tokens per batch
TQ = 512          # tokens owned by this core
P = 128
KO = D // P       # 8 feature chunks
RT_ALL = T // P   # 16 row tiles per batch
RT_OWN = TQ // P  # 4 row tiles owned
SO = T // P       # 16 key chunks
EPS = 1e-5
SCALE = 1.0 / 32.0  # D ** -0.5


def _layer_norm_rows(nc, sng, xrow_ap, stats_pool, y_out_ap):
    """Row-major LayerNorm core: y = (x - mean(x)) * rsqrt(var(x) + eps).

    xrow_ap: [128, 1024] f32 SBUF; y_out_ap: [128, 1024] (any dtype) SBUF.
    gamma/beta are NOT applied here (folded into the transpose evacuation).
    """
    stats = stats_pool.tile([P, 2, 6], F32, tag="bnstats")
    xg = xrow_ap.rearrange("p (g d) -> p g d", g=2)
    for g in range(2):
        nc.vector.bn_stats(out=stats[:, g, :], in_=xg[:, g, :])
    mv = stats_pool.tile([P, 2], F32, tag="bnaggr")
    nc.vector.bn_aggr(out=mv[:], in_=stats[:])
    rstd = stats_pool.tile([P, 1], F32, tag="rstd")
    # rstd = 1 / sqrt(var + eps)
    nc.scalar.activation(out=rstd[:], in_=mv[:, 1:2], func=AF.Sqrt,
                         bias=sng["eps"][:], scale=1.0)
    nc.vector.reciprocal(out=rstd[:], in_=rstd[:])
    nc.vector.tensor_scalar(
        out=y_out_ap, in0=xrow_ap, scalar1=mv[:, 0:1], scalar2=rstd[:],
        op0=ALU.subtract, op1=ALU.mult)


def build_kernel():
    nc = bacc.Bacc(None, target_bir_lowering=False, debug=False)

    xb = nc.dram_tensor("xb", [T, D], F32, kind="ExternalInput")
    wq = nc.dram_tensor("wq", [D, D], BF16, kind="ExternalInput")
    wk = nc.dram_tensor("wk", [D, D], BF16, kind="ExternalInput")
    wv = nc.dram_tensor("wv", [D, D], BF16, kind="ExternalInput")
    wo = nc.dram_tensor("wo", [D, D], BF16, kind="ExternalInput")
    w1 = nc.dram_tensor("w1", [D, D], BF16, kind="ExternalInput")
    w2 = nc.dram_tensor("w2", [D, D], BF16, kind="ExternalInput")
    cq = nc.dram_tensor("cq", [D], F32, kind="ExternalInput")
    ck = nc.dram_tensor("ck", [D], F32, kind="ExternalInput")
    bo = nc.dram_tensor("bo", [D], F32, kind="ExternalInput")
    b1 = nc.dram_tensor("b1", [D], F32, kind="ExternalInput")
    b2 = nc.dram_tensor("b2", [D], F32, kind="ExternalInput")
    out = nc.dram_tensor("out", [TQ, D], F32, kind="ExternalOutput")

    # per-feature params as [128, 8] (partition p, chunk o) for feature-major use
    def pm(dram_vec):
        return dram_vec.rearrange("(o p) -> p o", p=P)

    with tile.TileContext(nc) as tc:
        with (
            tc.tile_pool(name="singles", bufs=1) as singles,
            tc.tile_pool(name="persist", bufs=1) as persist,
            tc.tile_pool(name="hrow", bufs=3) as hrow_pool,
            tc.tile_pool(name="stats", bufs=4) as stats_pool,
            tc.tile_pool(name="wstrip", bufs=2) as wstrip_pool,
        ):
            # ---------------- setup ----------------
            sng = {}
            id_bf = singles.tile([P, P], BF16, name="id_bf")
            make_identity(nc, id_bf[:])
            id_f32 = singles.tile([P, P], F32, name="id_f32")
            make_identity(nc, id_f32[:])
            sng["eps"] = singles.tile([P, 1], F32, name="eps")
            nc.vector.memset(sng["eps"][:], EPS)
            ones64 = singles.tile([1, E], BF16, name="ones64")
            nc.vector.memset(ones64[:], 1.0)


            cq_pm = singles.tile([P, KO], F32, name="cq_pm")
            nc.sync.dma_start(cq_pm[:], pm(cq))
            ck_pm = singles.tile([P, KO], F32, name="ck_pm")
            nc.sync.dma_start(ck_pm[:], pm(ck))
            bo_pm = singles.tile([P, KO], F32, name="bo_pm")
            nc.sync.dma_start(bo_pm[:], pm(bo))
            bf1_pm = singles.tile([P, KO], F32, name="bf1_pm")
            nc.sync.dma_start(bf1_pm[:], pm(b1))
            bf2_pm = singles.tile([P, KO], F32, name="bf2_pm")
            nc.sync.dma_start(bf2_pm[:], pm(b2))

            # ---------------- persistent activations ----------------
            kT = persist.tile([P, KO, T], BF16, name="kT")          # 4 MB
            vP = persist.tile([P, SO // 2, 2, H, E + 1], FP8, name="vP")  # 2.08 MB
            qPack = persist.tile([P, KO, 2, TQ], BF16, name="qPack")  # 2 MB
            nc.vector.memset(qPack[:], 0.0)
            oT = persist.tile([P, KO, TQ], BF16, name="oT")         # 1 MB
            x1 = persist.tile([P, RT_OWN, D], F32, name="x1")       # 2 MB
            h2T = persist.tile([P, KO, TQ], BF16, name="h2T")       # 1 MB
            fT = persist.tile([P, KO, TQ], BF16, name="fT")         # 1 MB

            # ones column of vP (softmax denominator trick)
            nc.vector.memset(vP[:, :, :, :, E], 1.0)

            # ---------------- phase 1: LN1 over all rows -> hT ----------------
            ps_w_ctx = tc.tile_pool(name="ps_w", bufs=2, space="PSUM")
            ps_w = ps_w_ctx.__enter__()
            ps_tr_ctx = tc.tile_pool(name="ps_tr", bufs=2, space="PSUM")
            ps_tr = ps_tr_ctx.__enter__()
            ctx12 = tc.tile_pool(name="hTp", bufs=1)
            hT_pool = ctx12.__enter__()
            wsmall_ctx = tc.tile_pool(name="wsmall", bufs=3)
            wsmall_pool = wsmall_ctx.__enter__()
            xrow_ctx = tc.tile_pool(name="xrow", bufs=2)
            xrow_pool = xrow_ctx.__enter__()
            hT = hT_pool.tile([P, KO, T], BF16, name="hT")          # 4 MB
            for rt in range(RT_ALL):
                if rt < RT_OWN:
                    x_t = x1[:, rt, :]   # own rows: keep the raw x for residual
                    nc.sync.dma_start(x_t, xb[rt * P:(rt + 1) * P, :])
                else:
                    x_tile = xrow_pool.tile([P, D], F32, tag="xrow")
                    nc.sync.dma_start(x_tile[:], xb[rt * P:(rt + 1) * P, :])
                    x_t = x_tile[:]
                y_row = hrow_pool.tile([P, D], BF16, tag="hrow")
                _layer_norm_rows(nc, sng, x_t, stats_pool, y_row[:])
                # gamma/beta are folded into the weights host-side, so the
                # transpose evacuation is a plain copy (batched 2 chunks/op)
                trp = ps_tr.tile([P, KO, P], BF16, tag="tr")
                for ko in range(KO):
                    nc.tensor.transpose(trp[:, ko, :], y_row[:, ko * P:(ko + 1) * P], id_bf[:])
                nc.scalar.copy(out=hT[:, :, rt * P:(rt + 1) * P], in_=trp[:])

            # ---------------- phases 2+3: q/k/v interleaved with attention ----------------
            xrow_ctx.__exit__(None, None, None)
            ps_tr_ctx.__exit__(None, None, None)
            exps_ctx = tc.tile_pool(name="exps", bufs=6)
            exps_pool = exps_ctx.__enter__()
            evac_ctx = tc.tile_pool(name="evac", bufs=3)
            evac_pool = evac_ctx.__enter__()
            ps_qk_ctx = tc.tile_pool(name="ps_qk", bufs=2, space="PSUM")
            ps_qk = ps_qk_ctx.__enter__()
            ps_u_ctx = tc.tile_pool(name="ps_u", bufs=2, space="PSUM")
            ps_u = ps_u_ctx.__enter__()

            # Dense k/v matmuls are emitted right after each attention head
            # pair (lower scheduler priority), so the in-order PE fills
            # exp-latency gaps with dense work instead of idling.
            def emit_k(he):
                wk_he = wsmall_pool.tile([P, KO, P], BF16, tag="w_he", name="wk_he")
                nc.sync.dma_start(
                    wk_he[:], wk.rearrange("(o p) n -> p o n", p=P)[:, :, he * P:(he + 1) * P])
                for t in range(RT_ALL // 4):
                    psk = ps_w.tile([P, 512], F32, tag="ps_w", name="psk")
                    for ko in range(KO):
                        nc.tensor.matmul(
                            psk[:], wk_he[:, ko, :], hT[:, ko, t * 512:(t + 1) * 512],
                            start=(ko == 0), stop=(ko == KO - 1))
                    nc.vector.tensor_scalar_add(
                        out=kT[:, he, t * 512:(t + 1) * 512], in0=psk[:],
                        scalar1=ck_pm[:, he:he + 1])

            def emit_q(he):
                wq_he = wsmall_pool.tile([P, KO, P], BF16, tag="w_he", name="wq_he")
                nc.sync.dma_start(
                    wq_he[:], wq.rearrange("(o p) n -> p o n", p=P)[:, :, he * P:(he + 1) * P])
                psq = ps_w.tile([P, 512], F32, tag="ps_w", name="psq")
                for ko in range(KO):
                    nc.tensor.matmul(
                        psq[:], wq_he[:, ko, :], hT[:, ko, 0:TQ],
                        start=(ko == 0), stop=(ko == KO - 1))
                nc.vector.tensor_scalar_add(
                    out=qPack[0:E, he, 0, :], in0=psq[0:E, :],
                    scalar1=cq_pm[0:E, he:he + 1])
                nc.vector.tensor_scalar_add(
                    out=qPack[E:P, he, 1, :], in0=psq[E:P, :],
                    scalar1=cq_pm[E:P, he:he + 1])

            def emit_v_load(nh):
                wv_strip = wstrip_pool.tile([P, KO, 512], BF16, tag="wstrip", name="wv_strip")
                nc.sync.dma_start(
                    wv_strip[:], wv.rearrange("(o p) n -> p o n", p=P)[:, :, nh * 512:(nh + 1) * 512])
                return wv_strip

            def emit_v(nh, wv_strip=None, so_list=None):
                if wv_strip is None:
                    wv_strip = emit_v_load(nh)
                for so in (range(SO) if so_list is None else so_list):
                    psv = ps_w.tile([P, 512], F32, tag="ps_w", name="psv")
                    for ko in range(KO):
                        nc.tensor.matmul(
                            psv[:], hT[:, ko, so * P:(so + 1) * P], wv_strip[:, ko, :],
                            start=(ko == 0), stop=(ko == KO - 1))
                    nc.vector.tensor_copy(
                        out=vP[:, so // 2, so % 2, nh * 8:(nh + 1) * 8, 0:E],
                        in_=psv[:].rearrange("p (h e) -> p h e", e=E))

            def emit_attn(h):
                pbase = (h % 2) * E
                ko_h = h // 2
                psu = ps_u.tile([P, 512], F32, tag="ps_u", name="psu")
                for sp in range(SO // 2):
                    pss = ps_qk.tile([P, 2, 512], F32, tag="ps_qk", name="pss")
                    for j in range(2):
                        so = 2 * sp + j
                        # full-K stationary (FWL-eligible); the other head's
                        # rows meet zeros in the packed q, so the sum is exact
                        nc.tensor.matmul(
                            pss[:, j, :],
                            kT[:, ko_h, so * P:(so + 1) * P],
                            qPack[:, ko_h, h % 2, :],
                            start=True, stop=True)
                    es = exps_pool.tile([P, 2, 512], FP8, tag="exps", name="es")
                    nc.scalar.activation(out=es[:], in_=pss[:], func=AF.Exp, scale=SCALE)
                    # fp8 DoubleRow: virtual K=256 sums both key chunks at once
                    nc.tensor.matmul(
                        psu[0:E + 1, :], vP[:, sp, :, h, :], es[:],
                        start=(sp == 0), stop=(sp == SO // 2 - 1),
                        perf_mode=mybir.MatmulPerfMode.DoubleRow)
                dinv_f = stats_pool.tile([1, TQ], F32, tag="dinv_f", name="dinv_f")
                nc.vector.reciprocal(out=dinv_f[:], in_=psu[E:E + 1, :])
                dinv = stats_pool.tile([1, TQ], BF16, tag="dinv", name="dinv")
                nc.vector.tensor_copy(out=dinv[:], in_=dinv_f[:])
                psb = ps_u.tile([P, 512], F32, tag="ps_u", name="psb")
                nc.tensor.matmul(psb[0:E, :], ones64[:], dinv[:], start=True, stop=True)
                dbc = evac_pool.tile([E, 512], BF16, tag="dbc", name="dbc")
                nc.vector.tensor_copy(out=dbc[:], in_=psb[0:E, :])
                nc.vector.tensor_tensor(
                    out=oT[pbase:pbase + E, ko_h, :], in0=psu[0:E, :], in1=dbc[:],
                    op=ALU.mult)

            for he in range(KO):
                emit_q(he)
            emit_k(0)
            emit_v(0)
            for pair in range(KO):
                emit_attn(2 * pair)
                if pair + 1 < KO:
                    emit_k(pair + 1)
                emit_attn(2 * pair + 1)
                if pair == 2:
                    v1_strip = emit_v_load(1)
                    emit_v(1, v1_strip, list(range(0, 8)))
                elif pair == 3:
                    emit_v(1, v1_strip, list(range(8, SO)))


            ps_u_ctx.__exit__(None, None, None)
            ps_qk_ctx.__exit__(None, None, None)
            evac_ctx.__exit__(None, None, None)
            exps_ctx.__exit__(None, None, None)
            wsmall_ctx.__exit__(None, None, None)
            ctx12.__exit__(None, None, None)
            evac_ctx = tc.tile_pool(name="evac2", bufs=3)
            evac_pool = evac_ctx.__enter__()
            ps_tr_ctx = tc.tile_pool(name="ps_tr2", bufs=2, space="PSUM")
            ps_tr = ps_tr_ctx.__enter__()

            # ---------------- phase 4: Wo projection + residual + LN2 ----------------
            for half in range(2):
                wo_strip = wstrip_pool.tile([P, KO, 512], BF16, tag="wstrip")
                nc.sync.dma_start(
                    wo_strip[:], wo.rearrange("(o p) n -> p o n", p=P)[:, :, half * 512:(half + 1) * 512])
                for m in range(4):
                    mm = half * 4 + m
                    psy = ps_w.tile([P, 512], F32, tag="ps_w")
                    for ko in range(KO):
                        nc.tensor.matmul(
                            psy[:], wo_strip[:, ko, m * P:(m + 1) * P], oT[:, ko, :],
                            start=(ko == 0), stop=(ko == KO - 1))
                    ysb = evac_pool.tile([P, 512], F32, tag="ysb")
                    nc.vector.tensor_scalar_add(out=ysb[:], in0=psy[:], scalar1=bo_pm[:, mm:mm + 1])
                    trp = ps_tr.tile([P, RT_OWN, P], F32, tag="tr")
                    for rt in range(RT_OWN):
                        nc.tensor.transpose(trp[:, rt, :], ysb[:, rt * P:(rt + 1) * P], id_f32[:])
                    nc.vector.tensor_tensor(
                        out=x1[:, :, mm * P:(mm + 1) * P],
                        in0=x1[:, :, mm * P:(mm + 1) * P], in1=trp[:], op=ALU.add)

            for rt in range(RT_OWN):
                y_row = hrow_pool.tile([P, D], BF16, tag="hrow")
                _layer_norm_rows(nc, sng, x1[:, rt, :], stats_pool, y_row[:])
                trp = ps_tr.tile([P, KO, P], BF16, tag="tr2")
                for ko in range(KO):
                    nc.tensor.transpose(trp[:, ko, :], y_row[:, ko * P:(ko + 1) * P], id_bf[:])
                nc.scalar.copy(out=h2T[:, :, rt * P:(rt + 1) * P], in_=trp[:])

            # ---------------- phase 5: FFN ----------------
            for half in range(2):
                w1_strip = wstrip_pool.tile([P, KO, 512], BF16, tag="wstrip")
                nc.sync.dma_start(
                    w1_strip[:], w1.rearrange("(o p) n -> p o n", p=P)[:, :, half * 512:(half + 1) * 512])
                for m in range(4):
                    mm = half * 4 + m
                    psf = ps_w.tile([P, 512], F32, tag="ps_w")
                    for ko in range(KO):
                        nc.tensor.matmul(
                            psf[:], w1_strip[:, ko, m * P:(m + 1) * P], h2T[:, ko, :],
                            start=(ko == 0), stop=(ko == KO - 1))
                    # f = gelu(x + b1), fused bias via activation
                    nc.scalar.activation(out=fT[:, mm, :], in_=psf[:], func=AF.Gelu,
                                         bias=bf1_pm[:, mm:mm + 1], scale=1.0)
            for half in range(2):
                w2_strip = wstrip_pool.tile([P, KO, 512], BF16, tag="wstrip")
                nc.sync.dma_start(
                    w2_strip[:], w2.rearrange("(o p) n -> p o n", p=P)[:, :, half * 512:(half + 1) * 512])
                for m in range(4):
                    mm = half * 4 + m
                    psz = ps_w.tile([P, 512], F32, tag="ps_w")
                    for ko in range(KO):
                        nc.tensor.matmul(
                            psz[:], w2_strip[:, ko, m * P:(m + 1) * P], fT[:, ko, :],
                            start=(ko == 0), stop=(ko == KO - 1))
                    zsb = evac_pool.tile([P, 512], F32, tag="ysb")
                    nc.vector.tensor_scalar_add(out=zsb[:], in0=psz[:], scalar1=bf2_pm[:, mm:mm + 1])
                    trp = ps_tr.tile([P, RT_OWN, P], F32, tag="tr")
                    for rt in range(RT_OWN):
                        nc.tensor.transpose(trp[:, rt, :], zsb[:, rt * P:(rt + 1) * P], id_f32[:])
                    nc.vector.tensor_tensor(
                        out=x1[:, :, mm * P:(mm + 1) * P],
                        in0=x1[:, :, mm * P:(mm + 1) * P], in1=trp[:], op=ALU.add)

            for rt in range(RT_OWN):
                nc.sync.dma_start(out[rt * P:(rt + 1) * P, :], x1[:, rt, :])

            ps_tr_ctx.__exit__(None, None, None)
            evac_ctx.__exit__(None, None, None)
            ps_w_ctx.__exit__(None, None, None)

    nc.compile()
    return nc


_NC_CACHE = None


def _get_nc():
    global _NC_CACHE
    if _NC_CACHE is None:
        _NC_CACHE = build_kernel()
    return _NC_CACHE


def _prep_weights(Wq, Wk, Wv, Wo, W1, W2, ln1_g, ln1_b, ln2_g, ln2_b, b1):
    """Fold LayerNorm gamma into the consuming weights and beta into bias
    vectors (exact math, done in f32 before the bf16 cast)."""
    bf = ml_dtypes.bfloat16
    # [H, D, E] -> [D, H*E]
    wq = np.ascontiguousarray(np.transpose(Wq, (1, 0, 2)).reshape(D, D))
    wk = np.ascontiguousarray(np.transpose(Wk, (1, 0, 2)).reshape(D, D))
    wv = np.ascontiguousarray(np.transpose(Wv, (1, 0, 2)).reshape(D, D))
    cq = ln1_b @ wq
    ck = ln1_b @ wk
    cv = ln1_b @ wv              # v bias; o = softmax(..)@v + cv, folded into bo
    bo_adj = cv @ Wo             # caller adds this to bo
    b1_adj = b1 + ln2_b @ W1
    return ((wq * ln1_g[:, None]).astype(bf), (wk * ln1_g[:, None]).astype(bf),
            (wv * ln1_g[:, None]).astype(bf), Wo.astype(bf),
            (W1 * ln2_g[:, None]).astype(bf), W2.astype(bf),
            cq.astype(np.float32), ck.astype(np.float32),
            bo_adj.astype(np.float32), b1_adj.astype(np.float32))


def kernel(x, Wq, Wk, Wv, Wo, bo, ln1_g, ln1_b, ln2_g, ln2_b, W1, b1, W2, b2,
           _trace=False):
    x = np.asarray(x, dtype=np.float32)
    wq, wk, wv, wo, w1, w2, cq_v, ck_v, bo_extra, b1_adj = _prep_weights(
        np.asarray(Wq, np.float32), np.asarray(Wk, np.float32),
        np.asarray(Wv, np.float32), np.asarray(Wo, np.float32),
        np.asarray(W1, np.float32), np.asarray(W2, np.float32),
        np.asarray(ln1_g, np.float32), np.asarray(ln1_b, np.float32),
        np.asarray(ln2_g, np.float32), np.asarray(ln2_b, np.float32),
        np.asarray(b1, np.float32))
    common = {
        "wq": wq, "wk": wk, "wv": wv, "wo": wo, "w1": w1, "w2": w2,
        "cq": cq_v, "ck": ck_v,
        "bo": np.asarray(bo, np.float32) + bo_extra, "b1": b1_adj,
        "b2": np.asarray(b2, np.float32),
    }
    in_maps = []
    for core in range(8):
        b, c = divmod(core, 4)
        xb_perm = np.concatenate(
            [x[b, c * TQ:(c + 1) * TQ], x[b, :c * TQ], x[b, (c + 1) * TQ:]], axis=0)
        in_maps.append({"xb": np.ascontiguousarray(xb_perm), **common})

    nc = _get_nc()
    res = run_bass_kernel_spmd(nc, in_maps, core_ids=list(range(8)), trace=_trace)
    out = np.empty((2, T, D), np.float32)
    for core in range(8):
        b, c = divmod(core, 4)
        out[b, c * TQ:(c + 1) * TQ] = res.results[core]["out"]
    if _trace:
        kernel.last_results = res
    return out
